# revision 5
# baseline (speedup 1.0000x reference)
"""Trainium2 Bass kernel for a 3-layer GCN (JKNet, mode='cat') — 8-core SPMD.

v4 (dst-sharded, f16 pair-row gather at the DMA descriptor wall):
  - Nodes partitioned across 8 cores (6250 each, padded to 6272). Each core
    owns all edges whose destination lands in its range; nodes are
    (full-)degree-sorted so round s covers a dense prefix of node slots and
    the accumulate is a plain strided DVE add (no scatter anywhere).
  - Per layer: each core computes its slice of h @ W (node-major, wrapped),
    stages it in f16, an AllGather builds the full 50176x64 f16 table in
    DRAM. Gather rows are 256B PAIRS of adjacent wrapped rows, so the pair
    index (row>>1) covers the whole table in int16: ONE round system, no
    table-half split, no fold. A per-edge f16 weight pair (w,0)/(0,w)
    multiplies the gathered pair (inner-64 broadcast), zeroing the wrong
    node; the accumulator is double-wide [128, 49, 128] f16 and the two
    halves are added at finalize time.
  - Gather chunks of 2048 rows on 4 SWDGE queues: the ucode's engine hold
    backpressures at exactly the DMA engines' descriptor rate (~2ns/row
    measured), so desc-gen and transfers stay perfectly overlapped. Chunks
    cross round boundaries (per-round segs); interior round padding gathers
    row 0 with weight 0; a chunk's trailing pad is skipped via num_idxs_reg.
  - Tail per slot batch: halves-add + bias + ReLU, PE transpose to h^T for
    the next layer matmul and the JumpingKnowledge concat matmul.

Self-contained: hardcodes the problem geometry (N=50000, E=800000, 128->64,
3 layers, out 40) but computes all data-dependent schedules from the inputs.
"""

import sys

sys.path.insert(0, "/opt/trn_rl_repo")

import numpy as np

N = 50000
E = 800000
IN_DIM = 128
HID = 64
OUT_DIM = 40
M = 8
NPC = N // M
SLOTS = 49
SLICE = SLOTS * 128          # 6272
TABLE_ROWS = M * SLICE       # 50176
NPAIR = TABLE_ROWS // 2      # 25088 (int16-safe)
CMAX = 2048                  # gather rows per instruction
NQ = 4


def _wrap16(a):
    L = a.shape[0]
    return np.tile(a.reshape(L // 16, 16).T, (8, 1)).astype(np.int16)


def _rowof(q):
    """acc position q -> wrapped DRAM row (partition-major)."""
    return (q % 128) * SLOTS + q // 128


def _ranks_within(p):
    order = np.argsort(p, kind="stable")
    ps = p[order]
    starts = np.r_[0, np.nonzero(np.diff(ps))[0] + 1]
    counts = np.diff(np.r_[starts, len(ps)])
    r_sorted = np.arange(len(ps)) - np.repeat(starts, counts)
    r = np.empty_like(r_sorted)
    r[order] = r_sorted
    return r


def _plan_rounds(deg_lists):
    smax = int(max(int(d[0]) for d in deg_lists))
    widths, reals = [], []
    for s in range(smax):
        n_s = max(int((d > s).sum()) for d in deg_lists)
        if n_s == 0:
            break
        widths.append(((n_s + 127) // 128) * 128)
        reals.append(n_s)
    return widths, reals


def _chunk_plan(widths, reals, L):
    """Chunks of up to CMAX rows crossing round boundaries.
    Returns [(off, w, [(msg_slot, acc_slot, nslots)...], cnt)]."""
    roundoff = np.r_[0, np.cumsum(widths)].astype(np.int64)
    assert int(roundoff[-1]) == L
    chunks = []
    off = 0
    while off < L:
        w = min(CMAX, L - off)
        segs = []
        last_real = -1
        for s, w_s in enumerate(widths):
            a = max(off, int(roundoff[s]))
            b = min(off + w, int(roundoff[s + 1]))
            if a >= b:
                continue
            segs.append(((a - off) // 128, (a - int(roundoff[s])) // 128,
                         (b - a) // 128))
            re = min(b, int(roundoff[s]) + reals[s])
            if re > a:
                last_real = max(last_real, re)
        cnt = last_real - off
        assert cnt > 0
        chunks.append((off, w, segs, cnt))
        off += w
    return chunks


def _prep(x, edge_index, edge_weight):
    src = np.asarray(edge_index[0], dtype=np.int64)
    dst = np.asarray(edge_index[1], dtype=np.int64)
    ew = np.asarray(edge_weight, dtype=np.float32)
    x = np.asarray(x, dtype=np.float32)

    dcore = dst // NPC
    dloc = dst - dcore * NPC

    pos = np.empty(N, np.int64)
    pi_all = []
    deg_lists = []
    for c in range(M):
        deg = np.bincount(dloc[dcore == c], minlength=NPC)
        pi = np.argsort(-deg, kind="stable")
        p = np.empty(NPC, np.int64)
        p[pi] = np.arange(NPC)
        pos[c * NPC:(c + 1) * NPC] = p
        pi_all.append(pi)
        deg_lists.append(deg[pi])

    widths, reals = _plan_rounds(deg_lists)
    L = int(np.sum(widths))
    chunks = _chunk_plan(widths, reals, L)

    # global wrapped table row of a node
    table_row = (np.arange(N) // NPC) * SLICE + _rowof(pos)

    in_maps = []
    for c in range(M):
        mask = dcore == c
        tr = table_row[src[mask]]
        pidx = tr >> 1
        par = (tr & 1).astype(np.int64)
        roundoff = np.r_[0, np.cumsum(widths)]
        pd = pos[dst[mask]]
        r = _ranks_within(pd)
        flatpos = roundoff[r] + pd
        idx_flat = np.zeros(L, np.int64)
        ewp_flat = np.zeros((L, 2), np.float32)
        idx_flat[flatpos] = pidx
        ewp_flat[flatpos, par] = ew[mask]
        for (off, w, _s, cnt) in chunks:
            idx_flat[off + cnt:off + w] = -1

        # wrapped forms
        idx_w = _wrap16(idx_flat)
        ewp_w = np.ascontiguousarray(
            ewp_flat.reshape(L // 128, 128, 2).transpose(1, 0, 2)
        ).astype(np.float16)

        pi = pi_all[c]
        xT = np.zeros((IN_DIM, SLICE), np.float16)
        xT[:, :NPC] = x[c * NPC + pi, :].T

        in_maps.append({"xT": xT, "idx": idx_w, "ewp": ewp_w})

    plan = {"L": L, "widths": widths, "chunks": chunks, "pos": pos}
    return plan, in_maps


def _simulate(plan, in_maps, inputs):
    """Numpy model of the device schedule for validation (f32 math)."""
    x = np.asarray(inputs["x"], np.float32)
    Ws = [np.asarray(inputs[k], np.float32) for k in ("W1", "W2", "W3")]
    Wlin = np.asarray(inputs["Wlin"], np.float32)
    b = [np.asarray(inputs[k], np.float32) for k in ("b1", "b2", "b3")]
    blin = np.asarray(inputs["blin"], np.float32)
    chunks = plan["chunks"]
    pos = plan["pos"]
    L = plan["L"]

    # per-core wrapped node-major stripes [SLICE(wrapped rows), 64]
    def wrapped_rows(h_posorder):  # [SLICE, 64] position-ordered -> wrapped
        out = np.zeros((SLICE, HID), np.float32)
        q = np.arange(SLICE)
        out[_rowof(q)] = h_posorder
        return out

    stripes = []
    for c in range(M):
        xT = in_maps[c]["xT"].astype(np.float32)   # [128, SLICE] pos-ordered
        h = xT.T @ Ws[0]                           # [SLICE, 64]
        stripes.append(wrapped_rows(h))

    outs = np.empty((N, OUT_DIM), np.float32)
    h_layers = []
    for layer in range(3):
        table = np.concatenate(stripes, axis=0)    # [50176, 64]
        pairs = table.reshape(NPAIR, 128)
        h_new = []
        for c in range(M):
            im = in_maps[c]
            idx = im["idx"][:16].T.reshape(-1).astype(np.int64)
            ewp = im["ewp"].transpose(1, 0, 2).reshape(L, 2).astype(np.float32)
            acc = np.zeros((SLICE, 128), np.float32)
            msgbuf = np.zeros((CMAX, 128), np.float32)
            for (off, w, segs, cnt) in chunks:
                nproc = min((cnt + 127) // 128 * 128, w)
                ii = idx[off:off + nproc]
                valid = ii >= 0
                msgbuf[:nproc][valid] = pairs[ii[valid]]
                msgw = (msgbuf[:w].reshape(w, 2, 64)
                        * ewp[off:off + w, :, None]).reshape(w, 128)
                for (ms, ac, ns) in segs:
                    acc[ac * 128:(ac + ns) * 128] += msgw[ms * 128:(ms + ns) * 128]
            hsum = acc[:, 0:64] + acc[:, 64:128] + b[layer][None, :]
            h = np.maximum(hsum, 0.0)              # [SLICE(pos-order), 64]
            h_new.append(h)
        h_layers.append(h_new)
        if layer < 2:
            stripes = [wrapped_rows(h_new[c] @ Ws[layer + 1]) for c in range(M)]

    for c in range(M):
        hjk = np.concatenate([h_layers[0][c], h_layers[1][c], h_layers[2][c]],
                             axis=1)               # [SLICE, 192]
        oc = hjk @ Wlin + blin[None, :]
        q = pos[c * NPC:(c + 1) * NPC]
        outs[c * NPC:(c + 1) * NPC] = oc[q]
    return outs


def _build(plan, W1, b1, W2, b2, W3, b3, Wlin, blin):
    import concourse.bacc as bacc
    import concourse.mybir as mybir
    import concourse.tile as tile

    L = plan["L"]
    chunks = plan["chunks"]
    f32 = mybir.dt.float32
    f16 = mybir.dt.float16
    i16 = mybir.dt.int16

    nc = bacc.Bacc("TRN2", target_bir_lowering=False, debug=False,
                   num_devices=M, num_swdge_queues=NQ,
                   dynamic_dma_scratch_size=65536)

    xT_d = nc.dram_tensor("xT", [IN_DIM, SLICE], f16, kind="ExternalInput")
    idx_d = nc.dram_tensor("idx", [128, L // 16], i16, kind="ExternalInput")
    ewp_d = nc.dram_tensor("ewp", [128, L // 128, 2], f16, kind="ExternalInput")
    W1_d = nc.dram_tensor("W1", [IN_DIM, HID], f16, kind="ExternalInput")
    W2_d = nc.dram_tensor("W2", [HID, HID], f16, kind="ExternalInput")
    W3_d = nc.dram_tensor("W3", [128, HID], f16, kind="ExternalInput")  # rows 64-127
    Wl12_d = nc.dram_tensor("Wl12", [128, OUT_DIM], f16, kind="ExternalInput")
    Wl3_d = nc.dram_tensor("Wl3", [HID, OUT_DIM], f16, kind="ExternalInput")
    bias_d = nc.dram_tensor("bias", [128, 3 * HID], f16, kind="ExternalInput")
    blin_d = nc.dram_tensor("blin", [128, OUT_DIM], f32, kind="ExternalInput")
    out_d = nc.dram_tensor("out", [128, SLOTS, OUT_DIM], f32, kind="ExternalOutput")

    slice_d = nc.dram_tensor("slice_h16", [128, SLOTS, HID], f16)
    warm_d = nc.dram_tensor("warmt", [128, 128], f16)
    table_d = nc.dram_tensor("table16", [NPAIR, 128], f16, addr_space="Shared")

    qctr = [0]

    def nextq():
        q = qctr[0] % NQ
        qctr[0] += 1
        return q

    with tile.TileContext(nc) as tc:
        with (
            tc.tile_pool(name="const", bufs=1) as constp,
            tc.tile_pool(name="acc", bufs=1) as accp,
            tc.tile_pool(name="ht", bufs=1) as htp,
            tc.tile_pool(name="stag", bufs=1) as stagp,
            tc.tile_pool(name="msg", bufs=8) as msgp,
            tc.tile_pool(name="msgw", bufs=8) as msgwp,
            tc.tile_pool(name="warm", bufs=NQ) as warmp,
            tc.tile_pool(name="ps", bufs=3, space="PSUM") as psp,
            tc.tile_pool(name="pso", bufs=2, space="PSUM") as psop,
        ):
            xT = constp.tile([IN_DIM, SLICE], f16)
            idxt = constp.tile([128, L // 16], i16)
            ewpt = constp.tile([128, L // 128, 2], f16)
            W1t = constp.tile([IN_DIM, HID], f16)
            W2t = constp.tile([HID, HID], f16)
            W3t = constp.tile([128, HID], f16)
            Wl12t = constp.tile([128, OUT_DIM], f16)
            Wl3t = constp.tile([HID, OUT_DIM], f16)
            biast = constp.tile([128, 3 * HID], f16)
            blint = constp.tile([128, OUT_DIM], f32)
            ident = constp.tile([128, 128], f16)
            widx = constp.tile([128, 8], i16)

            for k in range(0, SLOTS, 7):
                cs = slice(k * 128, (k + 7) * 128)
                nc.sync.dma_start(xT[:, cs], xT_d[:, cs])
            for t, d in ((idxt, idx_d), (ewpt, ewp_d), (W1t, W1_d),
                         (W2t, W2_d), (W3t, W3_d), (Wl12t, Wl12_d),
                         (Wl3t, Wl3_d), (biast, bias_d), (blint, blin_d)):
                nc.sync.dma_start(t[:], d[:])
            from concourse.masks import make_identity
            make_identity(nc, ident[:])
            nc.vector.memset(widx[:], 0.0)

            for _ in range(8):
                wt = msgp.tile([128, CMAX // 128, 128], f16, tag="msg")
                nc.vector.memset(wt[:], 0.0)

            h12T = htp.tile([128, SLICE], f16)
            h3T = htp.tile([HID, SLICE], f16)

            relu = mybir.ActivationFunctionType.Relu
            copyf = mybir.ActivationFunctionType.Copy
            rfull = nc.gpsimd.to_reg(CMAX)

            # ---- layer-1 matmuls: x-slice @ W1 (node-major) ----
            stag = stagp.tile([128, SLOTS, HID], f16, tag="stag")
            for m in range(SLOTS):
                ps = psp.tile([128, HID], f32, tag="mm")
                nc.tensor.matmul(ps[:], xT[:, m * 128:(m + 1) * 128], W1t[:],
                                 start=True, stop=True)
                nc.scalar.activation(stag[:, m, :], ps[:], copyf)
            nc.sync.dma_start(slice_d[:], stag[:])

            ostag = stagp.tile([128, SLOTS, OUT_DIM], f32, tag="ostag")

            for layer in range(3):
                acc = accp.tile([128, SLOTS, 128], f16, tag="acc")
                nc.vector.memset(acc[:], 0.0)
                nc.gpsimd.collective_compute(
                    "AllGather", mybir.AluOpType.bypass,
                    replica_groups=[list(range(M))],
                    ins=[slice_d[:]], outs=[table_d[:]],
                )
                for _ in range(NQ):
                    wmsg = warmp.tile([128, 1, 128], f16, tag="wmsg")
                    nc.gpsimd.dma_gather(
                        wmsg[:, 0:1, :], warm_d[:], widx[:],
                        128, 128, 128, single_packet=False,
                        queue_num=nextq())

                if layer < 2:
                    stag = stagp.tile([128, SLOTS, HID], f16, tag="stag")

                bslice = biast[:, layer * HID:(layer + 1) * HID]

                def emit_tail(slots_list):
                    slots_list = sorted(slots_list)
                    ranges = []
                    for m in slots_list:
                        if ranges and ranges[-1][1] == m:
                            ranges[-1][1] = m + 1
                        else:
                            ranges.append([m, m + 1])
                    for (m0, m1) in ranges:
                        r = m1 - m0
                        # halves add + bias into the low half, then relu
                        nc.vector.tensor_add(acc[:, m0:m1, 0:HID],
                                             acc[:, m0:m1, 0:HID],
                                             acc[:, m0:m1, HID:128])
                        nc.vector.tensor_add(
                            acc[:, m0:m1, 0:HID], acc[:, m0:m1, 0:HID],
                            bslice.rearrange("p (s d) -> p s d", s=1)
                            .to_broadcast([128, r, HID]))
                        nc.scalar.activation(acc[:, m0:m1, 0:HID],
                                             acc[:, m0:m1, 0:HID], relu)
                    for (m0, m1) in ranges:
                        for m in range(m0, m1):
                            pst = psp.tile([HID, 128], f16, tag="tr")
                            nc.tensor.transpose(pst[:], acc[:, m, 0:HID],
                                                ident[:])
                            sl = slice(m * 128, (m + 1) * 128)
                            if layer == 0:
                                dst_ap = h12T[0:HID, sl]
                            elif layer == 1:
                                dst_ap = h12T[HID:128, sl]
                            else:
                                dst_ap = h3T[:, sl]
                            if m % 2 == 0:
                                nc.vector.tensor_copy(dst_ap, pst[:])
                            else:
                                nc.scalar.activation(dst_ap, pst[:], copyf)
                    for (m0, m1) in ranges:
                        for m in range(m0, m1):
                            sl = slice(m * 128, (m + 1) * 128)
                            if layer == 0:
                                ps = psp.tile([128, HID], f32, tag="mm")
                                nc.tensor.matmul(ps[:], h12T[0:HID, sl], W2t[:],
                                                 start=True, stop=True)
                                nc.scalar.activation(stag[:, m, :], ps[:], copyf)
                            elif layer == 1:
                                ps = psp.tile([128, HID], f32, tag="mm")
                                nc.tensor.matmul(ps[:], h12T[HID:128, sl],
                                                 W3t[HID:128, :],
                                                 start=True, stop=True)
                                nc.scalar.activation(stag[:, m, :], ps[:], copyf)
                            else:
                                pso = psop.tile([128, OUT_DIM], f32, tag="out")
                                nc.tensor.matmul(pso[:], h12T[:, sl],
                                                 Wl12t[:], start=True, stop=False)
                                nc.tensor.matmul(pso[:], h3T[:, sl],
                                                 Wl3t[:], start=False, stop=True)
                                nc.vector.tensor_add(ostag[:, m, :], pso[:],
                                                     blint[:])
                        if layer < 2:
                            nc.sync.dma_start(slice_d[:, m0:m1, :],
                                              stag[:, m0:m1, :])

                # finalization batches: slot m final after the last chunk
                # whose rounds reach it
                import numpy as _np
                widths = plan["widths"]
                roundoff = _np.r_[0, _np.cumsum(widths)].astype(_np.int64)
                batches = {}
                for m in range(SLOTS):
                    ss = [s for s in range(len(widths)) if widths[s] > 128 * m]
                    s_m = max(ss)
                    send = int(roundoff[s_m]) + min(128 * (m + 1),
                                                    widths[s_m]) - 1
                    for k, (off, w, _s, _c) in enumerate(chunks):
                        if off <= send < off + w:
                            batches.setdefault(k, []).append(m)
                            break

                for k, (off, w, segs, cnt) in enumerate(chunks):
                    ws = w // 128
                    msg = msgp.tile([128, CMAX // 128, 128], f16, tag="msg")
                    nreg = rfull if cnt == CMAX else cnt
                    nc.gpsimd.dma_gather(
                        msg[:, :ws, :], table_d[:],
                        idxt[:, off // 16:(off + w) // 16],
                        w, nreg, 128, single_packet=False, queue_num=nextq())
                    msgw = msgwp.tile([128, CMAX // 128, 128], f16, tag="msgw")
                    nc.vector.tensor_mul(
                        msgw[:, :ws, :].rearrange("p s (e d) -> p s e d", e=2),
                        msg[:, :ws, :].rearrange("p s (e d) -> p s e d", e=2),
                        ewpt[:, off // 128:(off + w) // 128, :]
                        .to_broadcast([128, ws, 2, HID]))
                    for (ms, as_, ns) in segs:
                        nc.vector.tensor_add(
                            acc[:, as_:as_ + ns, :], acc[:, as_:as_ + ns, :],
                            msgw[:, ms:ms + ns, :])
                    if k in batches:
                        emit_tail(batches[k])

            nc.sync.dma_start(out_d[:], ostag[:])

    nc.compile()
    return nc


_CACHE = {}


def kernel(x, edge_index, edge_weight, W1, b1, W2, b2, W3, b3, Wlin, blin):
    from concourse.bass_utils import run_bass_kernel_spmd

    x = np.asarray(x, dtype=np.float32)
    assert x.shape == (N, IN_DIM) and np.asarray(edge_index).shape == (2, E)

    key = hash(np.asarray(edge_index).tobytes())
    if key not in _CACHE:
        plan, in_maps = _prep(x, edge_index, edge_weight)
        nc = _build(plan, W1, b1, W2, b2, W3, b3, Wlin, blin)
        _CACHE[key] = (plan, nc)
    else:
        plan, nc = _CACHE[key]
        _, in_maps = _prep(x, edge_index, edge_weight)

    Wlin = np.asarray(Wlin, dtype=np.float32)
    shared = {
        "W1": np.asarray(W1, np.float16), "W2": np.asarray(W2, np.float16),
        "W3": np.concatenate([np.zeros((HID, HID), np.float16),
                              np.asarray(W3, np.float16)], axis=0),
        "Wl12": np.ascontiguousarray(Wlin[0:128]).astype(np.float16),
        "Wl3": np.ascontiguousarray(Wlin[128:192]).astype(np.float16),
        "bias": np.tile(np.concatenate(
            [np.asarray(b, np.float32) for b in (b1, b2, b3)])[None, :],
            (128, 1)).astype(np.float16),
        "blin": np.tile(np.asarray(blin, np.float32)[None, :], (128, 1)),
    }
    for im in in_maps:
        im.update(shared)

    res = run_bass_kernel_spmd(nc, in_maps, core_ids=list(range(M)))
    kernel._last_results = res
    kernel._last_in_maps = in_maps
    kernel._last_nc = nc

    pos = plan["pos"]
    out = np.empty((N, OUT_DIM), np.float32)
    for c in range(M):
        oc = res.results[c]["out"]  # [128, SLOTS, OUT]
        q = pos[c * NPC:(c + 1) * NPC]
        out[c * NPC:(c + 1) * NPC] = oc[q % 128, q // 128, :]
    return out
